# revision 1
# baseline (speedup 1.0000x reference)
"""MegablockMoE kernel for 8 Trainium2 NeuronCores.

Strategy (per sharding hint): expert-parallel. The router + token
dispatch/combine permutations (pure index bookkeeping + O(T*E*D) data
movement) run on host as the shard/unshard step; each of the 8 cores owns
one expert and runs the two big GEMMs (gelu(xg @ w1[e]) @ w2[e],
34.4 GFLOP/core) in bf16 with fp32 PSUM accumulation on its TensorEngine.

Device kernel (identical NEFF on all 8 cores, SPMD over experts):
    in : xgT [D, C] bf16   -- gathered tokens for this expert, transposed
         w1  [D, DFF] bf16, w2 [DFF, D] bf16
    mid: hT  [DFF, C] bf16 = gelu(w1.T @ xgT)     (exact erf gelu)
    out: yT  [D, C] f32    = w2.T @ hT
"""

import numpy as np
import ml_dtypes

import concourse.mybir as mybir
import concourse.tile as tile
from concourse import bacc
from concourse.bass_utils import run_bass_kernel_spmd
from concourse.kernels.tile_matmul import matmul_tile_kernel

B, S, D = 4, 2048, 1024
E, K, DFF = 8, 2, 4096
T = B * S
C = K * T // E  # 2048 expert capacity
BF16 = ml_dtypes.bfloat16
N_CORES = 8

_NC = None


def _build_nc():
    global _NC
    if _NC is not None:
        return _NC
    nc = bacc.Bacc(None, target_bir_lowering=False, debug=True)
    xgT = nc.dram_tensor("xgT", [D, C], mybir.dt.bfloat16, kind="ExternalInput")
    w1 = nc.dram_tensor("w1", [D, DFF], mybir.dt.bfloat16, kind="ExternalInput")
    w2 = nc.dram_tensor("w2", [DFF, D], mybir.dt.bfloat16, kind="ExternalInput")
    yT = nc.dram_tensor("yT", [D, C], mybir.dt.float32, kind="ExternalOutput")

    def gelu_evict(nc_, psum, sbuf):
        nc_.scalar.activation(sbuf, psum, mybir.ActivationFunctionType.Gelu)

    with tile.TileContext(nc) as tc:
        with tc.tile_pool(name="dram", bufs=1, space="DRAM") as dram:
            hT = dram.tile([DFF, C], mybir.dt.bfloat16)
            # hT = gelu(w1.T @ xgT)
            matmul_tile_kernel(tc, w1[:], xgT[:], hT[:], psum_evict_fn=gelu_evict)
            # yT = w2.T @ hT
            matmul_tile_kernel(tc, w2[:], hT[:], yT[:])
    nc.compile()
    _NC = nc
    return nc


def _route(x, wr):
    """Replicates the reference router exactly (fp32 numpy).

    Returns everything needed for dispatch + combine.
    """
    xt = np.transpose(x, (1, 0, 2)).reshape(T, D)  # [T, D] fp32
    logits = xt.astype(np.float32) @ wr.astype(np.float32)  # [T, E]
    m = logits.max(axis=-1, keepdims=True)
    p = np.exp(logits - m, dtype=np.float32)
    p /= p.sum(axis=-1, keepdims=True)
    top1 = np.argmax(p, axis=-1)
    p_masked = p.copy()
    p_masked[np.arange(T), top1] = -np.inf
    top2 = np.argmax(p_masked, axis=-1)
    eidx = np.stack([top1, top2], axis=1)  # [T, K]
    ew = np.take_along_axis(p, eidx, axis=1).astype(np.float32)  # [T, K]

    flat_e = eidx.reshape(-1)  # [T*K]
    order = np.argsort(flat_e, kind="stable")
    sorted_e = flat_e[order]
    hist = np.bincount(flat_e, minlength=E)
    starts = np.cumsum(hist) - hist
    pos = np.arange(T * K) - starts[sorted_e]
    keep = pos < C
    slot = np.where(keep, sorted_e * C + pos, E * C)
    token = order // K
    return xt, ew, order, keep, slot, token


def kernel(x, wr, w1, w2):
    nc = _build_nc()
    xt, ew, order, keep, slot, token = _route(x, wr)

    # --- dispatch: build per-expert gathered token buffers ---
    slot_token = np.zeros(E * C, np.int64)
    slot_token[slot[keep]] = token[keep]
    xT_bf = np.ascontiguousarray(xt.T.astype(BF16))  # [D, T]
    in_maps = []
    for e in range(E):
        idx = slot_token[e * C : (e + 1) * C]
        in_maps.append(
            {
                "xgT": np.ascontiguousarray(xT_bf[:, idx]),  # [D, C]
                "w1": np.ascontiguousarray(w1[e].astype(BF16)),
                "w2": np.ascontiguousarray(w2[e].astype(BF16)),
            }
        )

    res = run_bass_kernel_spmd(nc, in_maps, core_ids=list(range(N_CORES)))

    # --- combine: weighted scatter back to tokens ---
    Y = np.empty((E * C, D), np.float32)
    for e in range(E):
        Y[e * C : (e + 1) * C] = res.results[e]["yT"].T

    inv = np.empty(T * K, np.int64)
    inv[order] = np.arange(T * K)
    slot_tk = slot[inv].reshape(T, K)
    keep_tk = keep[inv].reshape(T, K)

    out_flat = np.zeros((T, D), np.float32)
    for k in range(K):
        sl = np.clip(slot_tk[:, k], 0, E * C - 1)
        contrib = Y[sl] * ew[:, k : k + 1]
        contrib[~keep_tk[:, k]] = 0.0
        out_flat += contrib
    return np.ascontiguousarray(
        out_flat.reshape(S, B, D).transpose(1, 0, 2)
    ).astype(np.float32)


# ---------------------------------------------------------------------------
# Benchmark helper (used by test.py; not part of the grading contract).
# Builds a persistent jitted callable mirroring bass2jax.run_bass_via_pjrt's
# multi-core path (no donation so buffers can be reused across timed calls).
# ---------------------------------------------------------------------------


def make_bench(in_maps):
    import jax
    from jax.experimental.shard_map import shard_map
    from jax.sharding import Mesh, PartitionSpec
    from concourse.bass2jax import (
        _bass_exec_p,
        install_neuronx_cc_hook,
        partition_id_tensor,
    )

    nc = _build_nc()
    install_neuronx_cc_hook()
    partition_name = nc.partition_id_tensor.name if nc.partition_id_tensor else None

    in_names, out_names, out_avals, zero_outs = [], [], [], []
    for alloc in nc.m.functions[0].allocations:
        if not isinstance(alloc, mybir.MemoryLocationSet):
            continue
        name = alloc.memorylocations[0].name
        if alloc.kind == "ExternalInput":
            if name != partition_name:
                in_names.append(name)
        elif alloc.kind == "ExternalOutput":
            shape = tuple(alloc.tensor_shape)
            dtype = mybir.dt.np(alloc.dtype)
            out_avals.append(jax.core.ShapedArray(shape, dtype))
            zero_outs.append(np.zeros(shape, dtype))
            out_names.append(name)
    n_params = len(in_names)
    all_in_names = list(in_names) + list(out_names)
    if partition_name is not None:
        all_in_names.append(partition_name)
    if nc.dbg_addr is not None:
        dbg_zero = np.zeros((1, 2), np.uint32)
        in_maps = [{**m, nc.dbg_addr.name: dbg_zero} for m in in_maps]

    def _body(*args):
        operands = list(args)
        if partition_name is not None:
            operands.append(partition_id_tensor())
        outs = _bass_exec_p.bind(
            *operands,
            out_avals=tuple(out_avals),
            in_names=tuple(all_in_names),
            out_names=tuple(out_names),
            lowering_input_output_aliases=(),
            sim_require_finite=True,
            sim_require_nnan=True,
            nc=nc,
        )
        return tuple(outs)

    devices = jax.devices()[:N_CORES]
    mesh = Mesh(np.asarray(devices), ("core",))
    n_outs = len(out_names)
    in_specs = (PartitionSpec("core"),) * (n_params + n_outs)
    out_specs = (PartitionSpec("core"),) * n_outs
    fn = jax.jit(
        shard_map(_body, mesh=mesh, in_specs=in_specs, out_specs=out_specs,
                  check_rep=False),
        keep_unused=True,
    )
    concat_in = [
        np.concatenate([np.asarray(in_maps[c][name]) for c in range(N_CORES)], axis=0)
        for name in in_names
    ]
    concat_zeros = [
        np.zeros((N_CORES * z.shape[0], *z.shape[1:]), z.dtype) for z in zero_outs
    ]
    args = [jax.device_put(a) for a in concat_in + concat_zeros]
    return fn, args, out_names


def benchmark(in_maps, iters=20, warmup=3):
    import time
    import jax

    fn, args, out_names = make_bench(in_maps)
    for _ in range(warmup):
        out = fn(*args)
        jax.block_until_ready(out)
    times = []
    for _ in range(iters):
        t0 = time.perf_counter()
        out = fn(*args)
        jax.block_until_ready(out)
        times.append(time.perf_counter() - t0)
    return min(times), sorted(times)[len(times) // 2], out


# revision 2
# speedup vs baseline: 1.3333x; 1.3333x over previous
"""MegablockMoE kernel for 8 Trainium2 NeuronCores.

Strategy (per sharding hint): expert-parallel. The router + token
dispatch/combine permutations (pure index bookkeeping + O(T*E*D) data
movement) run on host as the shard/unshard step; each of the 8 cores owns
one expert and runs the two big GEMMs (gelu(xg @ w1[e]) @ w2[e],
34.4 GFLOP/core) in bf16 with fp32 PSUM accumulation on its TensorEngine.

Device kernel (identical NEFF on all 8 cores, SPMD over experts):
    in : xgT [D, C] bf16   -- gathered tokens for this expert, transposed
         w1  [D, DFF] bf16, w2 [DFF, D] bf16
    mid: hT  [DFF, C] bf16 = gelu(w1.T @ xgT)     (exact erf gelu)
    out: yT  [D, C] f32    = w2.T @ hT
"""

import numpy as np
import ml_dtypes

import concourse.mybir as mybir
import concourse.tile as tile
from concourse import bacc
from concourse.bass_utils import run_bass_kernel_spmd
from concourse.kernels.tile_matmul import matmul_tile_kernel

B, S, D = 4, 2048, 1024
E, K, DFF = 8, 2, 4096
T = B * S
C = K * T // E  # 2048 expert capacity
BF16 = ml_dtypes.bfloat16
N_CORES = 8

_NC = None


def _build_nc():
    global _NC
    if _NC is not None:
        return _NC
    nc = bacc.Bacc(None, target_bir_lowering=False, debug=True)
    xgT = nc.dram_tensor("xgT", [D, C], mybir.dt.bfloat16, kind="ExternalInput")
    w1 = nc.dram_tensor("w1", [D, DFF], mybir.dt.bfloat16, kind="ExternalInput")
    w2 = nc.dram_tensor("w2", [DFF, D], mybir.dt.bfloat16, kind="ExternalInput")
    yT = nc.dram_tensor("yT", [D, C], mybir.dt.float32, kind="ExternalOutput")

    def gelu_evict(nc_, psum, sbuf):
        nc_.scalar.activation(sbuf, psum, mybir.ActivationFunctionType.Gelu)

    with tile.TileContext(nc) as tc:
        with tc.tile_pool(name="dram", bufs=1, space="DRAM") as dram:
            hT = dram.tile([DFF, C], mybir.dt.bfloat16)
            # hT = gelu(w1.T @ xgT)
            matmul_tile_kernel(tc, w1[:], xgT[:], hT[:], psum_evict_fn=gelu_evict)
            # yT = w2.T @ hT
            matmul_tile_kernel(tc, w2[:], hT[:], yT[:])
    nc.compile()
    _NC = nc
    return nc


def _route(x, wr):
    """Replicates the reference router exactly (fp32 numpy).

    Returns everything needed for dispatch + combine.
    """
    xt = np.transpose(x, (1, 0, 2)).reshape(T, D)  # [T, D] fp32
    logits = xt.astype(np.float32) @ wr.astype(np.float32)  # [T, E]
    m = logits.max(axis=-1, keepdims=True)
    p = np.exp(logits - m, dtype=np.float32)
    p /= p.sum(axis=-1, keepdims=True)
    top1 = np.argmax(p, axis=-1)
    p_masked = p.copy()
    p_masked[np.arange(T), top1] = -np.inf
    top2 = np.argmax(p_masked, axis=-1)
    eidx = np.stack([top1, top2], axis=1)  # [T, K]
    ew = np.take_along_axis(p, eidx, axis=1).astype(np.float32)  # [T, K]

    flat_e = eidx.reshape(-1)  # [T*K]
    order = np.argsort(flat_e, kind="stable")
    sorted_e = flat_e[order]
    hist = np.bincount(flat_e, minlength=E)
    starts = np.cumsum(hist) - hist
    pos = np.arange(T * K) - starts[sorted_e]
    keep = pos < C
    slot = np.where(keep, sorted_e * C + pos, E * C)
    token = order // K
    return xt, ew, order, keep, slot, token


def kernel(x, wr, w1, w2):
    nc = _build_nc()
    xt, ew, order, keep, slot, token = _route(x, wr)

    # --- dispatch: build per-expert gathered token buffers ---
    slot_token = np.zeros(E * C, np.int64)
    slot_token[slot[keep]] = token[keep]
    xT_bf = np.ascontiguousarray(xt.T.astype(BF16))  # [D, T]
    in_maps = []
    for e in range(E):
        idx = slot_token[e * C : (e + 1) * C]
        in_maps.append(
            {
                "xgT": np.ascontiguousarray(xT_bf[:, idx]),  # [D, C]
                "w1": np.ascontiguousarray(w1[e].astype(BF16)),
                "w2": np.ascontiguousarray(w2[e].astype(BF16)),
            }
        )

    res = run_bass_kernel_spmd(nc, in_maps, core_ids=list(range(N_CORES)))

    # --- combine: weighted scatter back to tokens ---
    Y = np.empty((E * C, D), np.float32)
    for e in range(E):
        Y[e * C : (e + 1) * C] = res.results[e]["yT"].T

    inv = np.empty(T * K, np.int64)
    inv[order] = np.arange(T * K)
    slot_tk = slot[inv].reshape(T, K)
    keep_tk = keep[inv].reshape(T, K)

    out_flat = np.zeros((T, D), np.float32)
    for k in range(K):
        sl = np.clip(slot_tk[:, k], 0, E * C - 1)
        contrib = Y[sl] * ew[:, k : k + 1]
        contrib[~keep_tk[:, k]] = 0.0
        out_flat += contrib
    return np.ascontiguousarray(
        out_flat.reshape(S, B, D).transpose(1, 0, 2)
    ).astype(np.float32)


# ---------------------------------------------------------------------------
# Benchmark helper (used by test.py; not part of the grading contract).
# Builds a persistent jitted callable mirroring bass2jax.run_bass_via_pjrt's
# multi-core path (no donation so buffers can be reused across timed calls).
# ---------------------------------------------------------------------------


def make_bench(in_maps):
    import jax
    from jax.experimental.shard_map import shard_map
    from jax.sharding import Mesh, PartitionSpec
    from concourse.bass2jax import (
        _bass_exec_p,
        install_neuronx_cc_hook,
        partition_id_tensor,
    )

    nc = _build_nc()
    install_neuronx_cc_hook()
    partition_name = nc.partition_id_tensor.name if nc.partition_id_tensor else None

    in_names, out_names, out_avals, zero_outs = [], [], [], []
    for alloc in nc.m.functions[0].allocations:
        if not isinstance(alloc, mybir.MemoryLocationSet):
            continue
        name = alloc.memorylocations[0].name
        if alloc.kind == "ExternalInput":
            if name != partition_name:
                in_names.append(name)
        elif alloc.kind == "ExternalOutput":
            shape = tuple(alloc.tensor_shape)
            dtype = mybir.dt.np(alloc.dtype)
            out_avals.append(jax.core.ShapedArray(shape, dtype))
            zero_outs.append(np.zeros(shape, dtype))
            out_names.append(name)
    n_params = len(in_names)
    all_in_names = list(in_names) + list(out_names)
    if partition_name is not None:
        all_in_names.append(partition_name)
    if nc.dbg_addr is not None:
        dbg_zero = np.zeros((1, 2), np.uint32)
        in_maps = [{**m, nc.dbg_addr.name: dbg_zero} for m in in_maps]

    def _body(*args):
        operands = list(args)
        if partition_name is not None:
            operands.append(partition_id_tensor())
        outs = _bass_exec_p.bind(
            *operands,
            out_avals=tuple(out_avals),
            in_names=tuple(all_in_names),
            out_names=tuple(out_names),
            lowering_input_output_aliases=(),
            sim_require_finite=True,
            sim_require_nnan=True,
            nc=nc,
        )
        return tuple(outs)

    devices = jax.devices()[:N_CORES]
    mesh = Mesh(np.asarray(devices), ("core",))
    n_outs = len(out_names)
    in_specs = (PartitionSpec("core"),) * (n_params + n_outs)
    out_specs = (PartitionSpec("core"),) * n_outs
    fn = jax.jit(
        shard_map(_body, mesh=mesh, in_specs=in_specs, out_specs=out_specs,
                  check_rep=False),
        keep_unused=True,
    )
    concat_in = [
        np.concatenate([np.asarray(in_maps[c][name]) for c in range(N_CORES)], axis=0)
        for name in in_names
    ]
    concat_zeros = [
        np.zeros((N_CORES * z.shape[0], *z.shape[1:]), z.dtype) for z in zero_outs
    ]
    from jax.sharding import NamedSharding

    shard = NamedSharding(mesh, PartitionSpec("core"))
    args = [jax.device_put(a, shard) for a in concat_in + concat_zeros]
    return fn, args, out_names


def benchmark(in_maps, iters=20, warmup=3):
    import time
    import jax

    fn, args, out_names = make_bench(in_maps)
    for _ in range(warmup):
        out = fn(*args)
        jax.block_until_ready(out)
    times = []
    for _ in range(iters):
        t0 = time.perf_counter()
        out = fn(*args)
        jax.block_until_ready(out)
        times.append(time.perf_counter() - t0)
    return min(times), sorted(times)[len(times) // 2], out
